# revision 1
# baseline (speedup 1.0000x reference)
"""DeepSpeed-style MLP block (residual-add + LayerNorm + GEMM + GeLU + GEMM +
residual) on 8 Trainium2 NeuronCores.

Sharding: data-parallel over tokens (B*S = 8192 -> 1024 tokens/core).  Each
core holds the full weights and computes its token slice end-to-end; no
collectives.  All matmuls run in fp32 on the PE (lhsT.T @ rhs, contraction on
the partition dim), so activations are transposed once after LayerNorm via PE
transposes ([H, tok] layout), the first GEMM produces h^T = [I, tok] tiles
(gelu applied on the PSUM->SBUF drain), and the second GEMM consumes h^T
chunks as the stationary operand against output_w rows, yielding out[tok, H]
naturally.
"""

import sys

sys.path.insert(0, "/opt/trn_rl_repo")

import numpy as np

try:
    import jax

    jax.config.update("jax_compilation_cache_dir", "/tmp/jax_neff_cache")
    jax.config.update("jax_persistent_cache_min_compile_time_secs", 1.0)
    jax.config.update("jax_persistent_cache_min_entry_size_bytes", 0)
except Exception:
    pass

import concourse.bass as bass  # noqa: F401  (engine types referenced via nc)
import concourse.mybir as mybir
from concourse import bacc
from concourse.masks import make_identity
from concourse.tile import TileContext

F32 = mybir.dt.float32
AF = mybir.ActivationFunctionType
N_CORES = 8
B, S, H, I = 4, 2048, 2048, 8192
LN_EPS = 1e-6
NTOK = B * S                 # 8192 tokens total
TLOC = NTOK // N_CORES       # 1024 tokens per core
TT = TLOC // 128             # 8 token tiles per core
HC = H // 128                # 16 eta (hidden) chunks
IC = I // 128                # 64 i chunks
OC = H // 512                # 4 output column chunks of 512

_CACHE = {}


def _build_program():
    nc = bacc.Bacc("TRN2", target_bir_lowering=False, debug=False,
                   num_devices=N_CORES)

    xin = nc.declare_dram_parameter("xin", [TLOC, H], F32, isOutput=False)
    xres = nc.declare_dram_parameter("xres", [TLOC, H], F32, isOutput=False)
    # w1p[i, c, p, f] = inter_w[c*128 + p, i*128 + f]
    w1p = nc.declare_dram_parameter("w1p", [IC, HC, 128, 128], F32, isOutput=False)
    w2p = nc.declare_dram_parameter("w2p", [I, H], F32, isOutput=False)
    biasb = nc.declare_dram_parameter("biasb", [128, H], F32, isOutput=False)
    obb = nc.declare_dram_parameter("obb", [128, H], F32, isOutput=False)
    gammat = nc.declare_dram_parameter("gammat", [128, HC], F32, isOutput=False)
    betat = nc.declare_dram_parameter("betat", [128, HC], F32, isOutput=False)
    ibt = nc.declare_dram_parameter("ibt", [128, IC], F32, isOutput=False)
    out = nc.declare_dram_parameter("out", [TLOC, H], F32, isOutput=True)

    with TileContext(nc) as tc:
        with (
            tc.tile_pool(name="perm", bufs=1) as perm,
            tc.tile_pool(name="dram", bufs=1, space="DRAM") as dpool,
        ):
            ident = perm.tile([128, 128], F32)
            make_identity(nc, ident[:])
            eps = perm.tile([128, 1], F32)
            nc.vector.memset(eps[:], LN_EPS)
            gam = perm.tile([128, HC], F32)
            bet = perm.tile([128, HC], F32)
            ib = perm.tile([128, IC], F32)
            nc.sync.dma_start(out=gam[:], in_=gammat[:])
            nc.sync.dma_start(out=bet[:], in_=betat[:])
            nc.sync.dma_start(out=ib[:], in_=ibt[:])

            # residual_add tiles stay resident for the final add
            ras = [perm.tile([128, H], F32, name=f"ra{t}") for t in range(TT)]
            hts_dram = [dpool.tile([128, TLOC], F32, name=f"htd{i}")
                        for i in range(IC)]

            with tc.tile_pool(name="p12", bufs=1) as p12:
                # ln^T resident: one [128, TLOC] tile per eta chunk
                lnt = [p12.tile([128, TLOC], F32, name=f"lnt{c}")
                       for c in range(HC)]

                # ---------------- phase 1: residual add + LN + transpose ----
                with (
                    tc.tile_pool(name="p1", bufs=2) as p1,
                    tc.tile_pool(name="p1c", bufs=1) as p1c,
                    tc.tile_pool(name="trp", bufs=4, space="PSUM") as trp,
                ):
                    bb = p1c.tile([128, H], F32)
                    nc.sync.dma_start(out=bb[:], in_=biasb[:])
                    for t in range(TT):
                        tin = p1.tile([128, H], F32, tag="tin")
                        tre = p1.tile([128, H], F32, tag="tre")
                        nc.sync.dma_start(out=tin[:], in_=xin[t * 128:(t + 1) * 128, :])
                        nc.sync.dma_start(out=tre[:], in_=xres[t * 128:(t + 1) * 128, :])
                        ra = ras[t]
                        nc.vector.tensor_add(ra[:], tin[:], tre[:])
                        nc.vector.tensor_add(ra[:], ra[:], bb[:])
                        # stats
                        scr = p1.tile([128, H], F32, tag="scr")
                        ssq = p1.tile([128, 1], F32, tag="ssq")
                        nc.scalar.activation(scr[:], ra[:], AF.Square,
                                             accum_out=ssq[:])
                        s1 = p1.tile([128, 1], F32, tag="s1")
                        nc.vector.reduce_sum(s1[:], ra[:], axis=mybir.AxisListType.X)
                        mu = p1.tile([128, 1], F32, tag="mu")
                        nc.vector.tensor_scalar_mul(mu[:], s1[:], 1.0 / H)
                        ex2 = p1.tile([128, 1], F32, tag="ex2")
                        nc.vector.tensor_scalar_mul(ex2[:], ssq[:], 1.0 / H)
                        mu2 = p1.tile([128, 1], F32, tag="mu2")
                        nc.vector.tensor_mul(mu2[:], mu[:], mu[:])
                        var = p1.tile([128, 1], F32, tag="var")
                        nc.vector.tensor_sub(var[:], ex2[:], mu2[:])
                        std = p1.tile([128, 1], F32, tag="std")
                        nc.scalar.activation(std[:], var[:], AF.Sqrt, bias=eps[:])
                        rstd = p1.tile([128, 1], F32, tag="rstd")
                        nc.vector.reciprocal(rstd[:], std[:])
                        z = p1.tile([128, H], F32, tag="scr")
                        nc.vector.tensor_scalar(
                            z[:], ra[:], mu[:], rstd[:],
                            op0=mybir.AluOpType.subtract,
                            op1=mybir.AluOpType.mult,
                        )
                        # transpose 128x128 blocks; gamma/beta on the drain
                        for c in range(HC):
                            ps = trp.tile([128, 128], F32, tag="tr")
                            nc.tensor.transpose(
                                ps[:], z[:, c * 128:(c + 1) * 128], ident[:])
                            nc.scalar.activation(
                                lnt[c][:, t * 128:(t + 1) * 128], ps[:],
                                AF.Identity,
                                bias=bet[:, c:c + 1], scale=gam[:, c:c + 1])

                # ---------------- phase 2: h^T = gelu(W1^T @ ln^T + b1) -----
                with (
                    tc.tile_pool(name="p2", bufs=3) as p2,
                    tc.tile_pool(name="pshp", bufs=4, space="PSUM") as pshp,
                ):
                    for i in range(IC):
                        w1t = p2.tile([128, H], F32, tag="w1t")
                        for c in range(HC):
                            nc.sync.dma_start(
                                out=w1t[:, c * 128:(c + 1) * 128],
                                in_=w1p[i, c])
                        ht = p2.tile([128, TLOC], F32, tag="ht")
                        for half in range(TLOC // 512):
                            psh = pshp.tile([128, 512], F32, tag="psh")
                            for c in range(HC):
                                nc.tensor.matmul(
                                    psh[:],
                                    w1t[:, c * 128:(c + 1) * 128],
                                    lnt[c][:, half * 512:(half + 1) * 512],
                                    start=(c == 0), stop=(c == HC - 1))
                            nc.scalar.activation(
                                ht[:, half * 512:(half + 1) * 512], psh[:],
                                AF.Gelu, bias=ib[:, i:i + 1])
                        nc.sync.dma_start(out=hts_dram[i][:], in_=ht[:])

            # ---------------- phase 3: out = h @ W2 + ra + b_out ------------
            with (
                tc.tile_pool(name="p3", bufs=1) as p3c,
                tc.tile_pool(name="p3w", bufs=6) as p3w,
                tc.tile_pool(name="p3h", bufs=10) as p3h,
                tc.tile_pool(name="p3o", bufs=3) as p3o,
                tc.tile_pool(name="psop", bufs=1, space="PSUM") as psop,
            ):
                ob = p3c.tile([128, H], F32)
                nc.sync.dma_start(out=ob[:], in_=obb[:])
                for pair in range(TT // 2):
                    psos = [psop.tile([128, H], F32, name=f"pso{pair}_{k}",
                                      tag=f"pso{k}") for k in range(2)]
                    for i in range(IC):
                        w2t = p3w.tile([128, H], F32, tag="w2t")
                        nc.sync.dma_start(
                            out=w2t[:], in_=w2p[i * 128:(i + 1) * 128, :])
                        for k in range(2):
                            t = pair * 2 + k
                            htt = p3h.tile([128, 128], F32, tag="htt")
                            nc.sync.dma_start(
                                out=htt[:],
                                in_=hts_dram[i][:, t * 128:(t + 1) * 128])
                            for o in range(OC):
                                nc.tensor.matmul(
                                    psos[k][:, o * 512:(o + 1) * 512],
                                    htt[:],
                                    w2t[:, o * 512:(o + 1) * 512],
                                    start=(i == 0), stop=(i == IC - 1))
                    for k in range(2):
                        t = pair * 2 + k
                        osb = p3o.tile([128, H], F32, tag="osb")
                        nc.vector.tensor_add(osb[:], psos[k][:], ras[t][:])
                        nc.vector.tensor_add(osb[:], osb[:], ob[:])
                        nc.sync.dma_start(
                            out=out[t * 128:(t + 1) * 128, :], in_=osb[:])

    nc.compile()
    return nc


def _get_program():
    if "nc" not in _CACHE:
        _CACHE["nc"] = _build_program()
    return _CACHE["nc"]


def kernel(input, residual, residual_norm, bias, gamma, beta,
           inter_w, inter_b, output_w, output_b):
    nc = _get_program()

    input = np.ascontiguousarray(np.asarray(input, dtype=np.float32))
    residual = np.ascontiguousarray(np.asarray(residual, dtype=np.float32))
    bias = np.asarray(bias, dtype=np.float32)
    gamma = np.asarray(gamma, dtype=np.float32)
    beta = np.asarray(beta, dtype=np.float32)
    inter_w = np.asarray(inter_w, dtype=np.float32)
    inter_b = np.asarray(inter_b, dtype=np.float32)
    output_w = np.ascontiguousarray(np.asarray(output_w, dtype=np.float32))
    output_b = np.asarray(output_b, dtype=np.float32)

    xin = input.reshape(NTOK, H)
    xres = residual.reshape(NTOK, H)
    # w1p[i, c, p, f] = inter_w[c*128+p, i*128+f]
    w1p = np.ascontiguousarray(
        inter_w.reshape(HC, 128, IC, 128).transpose(2, 0, 1, 3))
    biasb = np.ascontiguousarray(np.broadcast_to(bias, (128, H)))
    obb = np.ascontiguousarray(np.broadcast_to(output_b, (128, H)))
    gammat = np.ascontiguousarray(gamma.reshape(HC, 128).T)
    betat = np.ascontiguousarray(beta.reshape(HC, 128).T)
    ibt = np.ascontiguousarray(inter_b.reshape(IC, 128).T)

    in_maps = []
    for c in range(N_CORES):
        in_maps.append({
            "xin": np.ascontiguousarray(xin[c * TLOC:(c + 1) * TLOC]),
            "xres": np.ascontiguousarray(xres[c * TLOC:(c + 1) * TLOC]),
            "w1p": w1p,
            "w2p": output_w,
            "biasb": biasb,
            "obb": obb,
            "gammat": gammat,
            "betat": betat,
            "ibt": ibt,
        })

    from concourse.bass_utils import run_bass_kernel_spmd
    res = run_bass_kernel_spmd(nc, in_maps, list(range(N_CORES)))
    out = np.concatenate([res.results[c]["out"] for c in range(N_CORES)], axis=0)
    return out.reshape(B, S, H)


if __name__ == "__main__":
    nc = _get_program()
    from concourse.timeline_sim import TimelineSim
    ts = TimelineSim(nc)
    total = ts.simulate()
    print(f"TimelineSim: {total:.0f} ns")



# revision 3
# speedup vs baseline: 3.8543x; 3.8543x over previous
"""DeepSpeed-style MLP block (residual-add + LayerNorm + GEMM + GeLU + GEMM +
residual) on 8 Trainium2 NeuronCores.

Sharding: data-parallel over tokens (B*S = 8192 -> 1024 tokens/core).  Each
core holds the full weights and computes its token slice end-to-end; no
collectives.

All matmuls run in bf16 on the PE (1 cycle/row vs fp32's 4) with fp32 PSUM
accumulation.  Per core the tokens are processed as two 512-token groups:
GEMM1 (64 rank-128 i-chunks) produces h^T tiles [128, 512] that stay resident
in SBUF, then GEMM2 accumulates over all 64 i-chunks into PSUM for 4 output
column chunks of 512.  Weights stream from DRAM twice (once per group), which
the DMA engines hide entirely under the PE's compute.

LayerNorm statistics use bn_stats/bn_aggr on the vector engine; the
normalized activations are transposed to [H, tok] via PE transposes with
gamma/beta applied on the PSUM->SBUF drain.
"""

import sys

sys.path.insert(0, "/opt/trn_rl_repo")

import numpy as np

try:
    import jax

    jax.config.update("jax_compilation_cache_dir", "/tmp/jax_neff_cache")
    jax.config.update("jax_persistent_cache_min_compile_time_secs", 1.0)
    jax.config.update("jax_persistent_cache_min_entry_size_bytes", 0)
except Exception:
    pass

import ml_dtypes

import concourse.bass as bass  # noqa: F401
import concourse.mybir as mybir
from concourse import bacc
from concourse.masks import make_identity
from concourse.tile import TileContext

F32 = mybir.dt.float32
BF16 = mybir.dt.bfloat16
AF = mybir.ActivationFunctionType
ALU = mybir.AluOpType
NP_BF16 = ml_dtypes.bfloat16

N_CORES = 8
B, S, H, I = 4, 2048, 2048, 8192
LN_EPS = 1e-6
NTOK = B * S                 # 8192 tokens total
TLOC = NTOK // N_CORES       # 1024 tokens per core
TT = TLOC // 128             # 8 token tiles per core
HC = H // 128                # 16 hidden chunks (contraction for GEMM1)
IC = I // 128                # 64 intermediate chunks
GROUPS = 2                   # token groups per core
GT = TT // GROUPS            # 4 token tiles per group
GTOK = TLOC // GROUPS        # 512 tokens per group
OC = H // 512                # 4 output column chunks of 512

_CACHE = {}


def _build_program():
    nc = bacc.Bacc("TRN2", target_bir_lowering=False, debug=False,
                   num_devices=N_CORES)

    xin = nc.declare_dram_parameter("xin", [TLOC, H], BF16, isOutput=False)
    xres = nc.declare_dram_parameter("xres", [TLOC, H], BF16, isOutput=False)
    # w1p[i, p, c*128 + f] = inter_w[c*128 + p, i*128 + f]
    w1p = nc.declare_dram_parameter("w1p", [IC, 128, H], BF16, isOutput=False)
    w2p = nc.declare_dram_parameter("w2p", [I, H], BF16, isOutput=False)
    bbt = nc.declare_dram_parameter("bbt", [128, H], BF16, isOutput=False)
    obt = nc.declare_dram_parameter("obt", [128, H], BF16, isOutput=False)
    gammat = nc.declare_dram_parameter("gammat", [128, HC], F32, isOutput=False)
    betat = nc.declare_dram_parameter("betat", [128, HC], F32, isOutput=False)
    ibt = nc.declare_dram_parameter("ibt", [128, IC], F32, isOutput=False)
    out = nc.declare_dram_parameter("out", [TLOC, H], F32, isOutput=True)

    with TileContext(nc) as tc:
        with (
            tc.tile_pool(name="perm", bufs=1) as perm,
            tc.tile_pool(name="p1", bufs=2) as p1,
            tc.tile_pool(name="w1pool", bufs=3) as w1pool,
            tc.tile_pool(name="w2pool", bufs=6) as w2pool,
            tc.tile_pool(name="htpool", bufs=IC) as htpool,
            tc.tile_pool(name="osbp", bufs=3) as osbp,
            tc.tile_pool(name="ps", bufs=1, space="PSUM") as ps,
        ):
            ident = perm.tile([128, 128], BF16)
            make_identity(nc, ident[:])
            eps = perm.tile([128, 1], F32)
            nc.vector.memset(eps[:], LN_EPS)
            gam = perm.tile([128, HC], F32)
            bet = perm.tile([128, HC], F32)
            ib = perm.tile([128, IC], F32)
            bb = perm.tile([128, H], BF16)
            ob = perm.tile([128, H], BF16)
            nc.sync.dma_start(out=gam[:], in_=gammat[:])
            nc.sync.dma_start(out=bet[:], in_=betat[:])
            nc.sync.dma_start(out=ib[:], in_=ibt[:])
            nc.sync.dma_start(out=bb[:], in_=bbt[:])
            nc.sync.dma_start(out=ob[:], in_=obt[:])

            # persistent per-core activations
            lnt = [perm.tile([128, TLOC], BF16, name=f"lnt{c}")
                   for c in range(HC)]
            rao = [perm.tile([128, H], BF16, name=f"rao{t}")
                   for t in range(TT)]

            zs = {}

            def p1_compute(t):
                """residual add + LN stats + normalize for token tile t."""
                tin = p1.tile([128, H], BF16, tag="tin")
                tre = p1.tile([128, H], BF16, tag="tre")
                nc.sync.dma_start(out=tin[:], in_=xin[t * 128:(t + 1) * 128, :])
                nc.sync.dma_start(out=tre[:], in_=xres[t * 128:(t + 1) * 128, :])
                ra = p1.tile([128, H], BF16, tag="ra")
                nc.vector.tensor_add(ra[:], tin[:], tre[:])
                nc.vector.tensor_add(ra[:], ra[:], bb[:])
                # final-residual term (ra + output_b) off the critical path
                nc.gpsimd.tensor_add(rao[t][:], ra[:], ob[:])
                # mean/var via bn_stats over 4 chunks of 512
                stats = p1.tile([128, 4, 6], F32, tag="stats")
                rav = ra[:].rearrange("p (n f) -> p n f", f=512)
                for sub in range(4):
                    nc.vector.bn_stats(stats[:, sub, :], rav[:, sub, :])
                mv = p1.tile([128, 2], F32, tag="mv")
                nc.vector.bn_aggr(mv[:], stats[:])
                std = p1.tile([128, 1], F32, tag="std")
                nc.scalar.activation(std[:], mv[:, 1:2], AF.Sqrt, bias=eps[:])
                rstd = p1.tile([128, 1], F32, tag="rstd")
                nc.vector.reciprocal(rstd[:], std[:])
                z = p1.tile([128, H], BF16, tag="z", bufs=4)
                nc.vector.tensor_scalar(
                    z[:], ra[:], mv[:, 0:1], rstd[:],
                    op0=ALU.subtract, op1=ALU.mult)
                zs[t] = z

            def p1_transpose(t):
                """z[tok, H] -> lnt[c][H-sub, tok] with gamma/beta on drain."""
                z = zs[t]
                for c in range(HC):
                    tr = ps.tile([128, 128], BF16, tag="trp", bufs=2)
                    nc.tensor.transpose(
                        tr[:], z[:, c * 128:(c + 1) * 128], ident[:])
                    nc.scalar.activation(
                        lnt[c][:, t * 128:(t + 1) * 128], tr[:],
                        AF.Identity,
                        bias=bet[:, c:c + 1], scale=gam[:, c:c + 1])

            hts = [[None] * IC for _ in range(GROUPS)]

            def g1_chunk(g, i):
                """h^T[i-block] = gelu(W1^T @ ln^T + b1) for group g."""
                w1t = w1pool.tile([128, H], BF16, tag="w1t")
                nc.sync.dma_start(out=w1t[:], in_=w1p[i])
                psh = ps.tile([128, GTOK], F32, tag="psh", bufs=2)
                for c in range(HC):
                    nc.tensor.matmul(
                        psh[:],
                        w1t[:, c * 128:(c + 1) * 128],
                        lnt[c][:, g * GTOK:(g + 1) * GTOK],
                        start=(c == 0), stop=(c == HC - 1))
                ht = htpool.tile([128, GTOK], BF16, tag="ht")
                nc.scalar.activation(ht[:], psh[:], AF.Gelu,
                                     bias=ib[:, i:i + 1])
                hts[g][i] = ht

            def g2_group(g):
                """out[group tokens] = h @ W2 + (ra + output_b)."""
                for oc in range(OC):
                    psos = [ps.tile([128, 512], F32, name=f"pso_{g}_{oc}_{t}",
                                    tag=f"pso{t}", bufs=1) for t in range(GT)]
                    for i in range(IC):
                        w2c = w2pool.tile([128, 512], BF16, tag="w2c")
                        nc.scalar.dma_start(
                            out=w2c[:],
                            in_=w2p[i * 128:(i + 1) * 128,
                                    oc * 512:(oc + 1) * 512])
                        ht = hts[g][i]
                        for t in range(GT):
                            nc.tensor.matmul(
                                psos[t][:],
                                ht[:, t * 128:(t + 1) * 128],
                                w2c[:],
                                start=(i == 0), stop=(i == IC - 1))
                    for t in range(GT):
                        tt = g * GT + t
                        osb = osbp.tile([128, 512], F32, tag="osb")
                        nc.vector.tensor_add(
                            osb[:], psos[t][:],
                            rao[tt][:, oc * 512:(oc + 1) * 512])
                        nc.sync.dma_start(
                            out=out[tt * 128:(tt + 1) * 128,
                                    oc * 512:(oc + 1) * 512],
                            in_=osb[:])

            # ---- emission order: pipeline phase 1 under GEMM1 of group 0 ----
            p1_compute(0)
            p1_compute(1)
            p1_transpose(0)
            p1_compute(2)
            p1_transpose(1)
            p1_compute(3)
            p1_transpose(2)
            p1_transpose(3)
            for t in range(4, TT):
                p1_compute(t)

            for i in range(0, 16):
                g1_chunk(0, i)
            p1_transpose(4)
            for i in range(16, 24):
                g1_chunk(0, i)
            p1_transpose(5)
            for i in range(24, 32):
                g1_chunk(0, i)
            p1_transpose(6)
            for i in range(32, 40):
                g1_chunk(0, i)
            p1_transpose(7)
            for i in range(40, IC):
                g1_chunk(0, i)

            g2_group(0)
            for i in range(IC):
                g1_chunk(1, i)
            g2_group(1)

    nc.compile()
    return nc


def _get_program():
    if "nc" not in _CACHE:
        _CACHE["nc"] = _build_program()
    return _CACHE["nc"]


def kernel(input, residual, residual_norm, bias, gamma, beta,
           inter_w, inter_b, output_w, output_b):
    nc = _get_program()

    input = np.asarray(input, dtype=np.float32)
    residual = np.asarray(residual, dtype=np.float32)
    bias = np.asarray(bias, dtype=np.float32)
    gamma = np.asarray(gamma, dtype=np.float32)
    beta = np.asarray(beta, dtype=np.float32)
    inter_w = np.asarray(inter_w, dtype=np.float32)
    inter_b = np.asarray(inter_b, dtype=np.float32)
    output_w = np.asarray(output_w, dtype=np.float32)
    output_b = np.asarray(output_b, dtype=np.float32)

    xin = np.ascontiguousarray(input.reshape(NTOK, H).astype(NP_BF16))
    xres = np.ascontiguousarray(residual.reshape(NTOK, H).astype(NP_BF16))
    # w1p[i, p, c*128+f] = inter_w[c*128+p, i*128+f]
    w1p = np.ascontiguousarray(
        inter_w.reshape(HC, 128, IC, 128).transpose(2, 1, 0, 3)
        .reshape(IC, 128, H).astype(NP_BF16))
    w2p = np.ascontiguousarray(output_w.astype(NP_BF16))
    bbt = np.ascontiguousarray(
        np.broadcast_to(bias.astype(NP_BF16), (128, H)))
    obt = np.ascontiguousarray(
        np.broadcast_to(output_b.astype(NP_BF16), (128, H)))
    gammat = np.ascontiguousarray(gamma.reshape(HC, 128).T)
    betat = np.ascontiguousarray(beta.reshape(HC, 128).T)
    ibt = np.ascontiguousarray(inter_b.reshape(IC, 128).T)

    in_maps = []
    for c in range(N_CORES):
        in_maps.append({
            "xin": np.ascontiguousarray(xin[c * TLOC:(c + 1) * TLOC]),
            "xres": np.ascontiguousarray(xres[c * TLOC:(c + 1) * TLOC]),
            "w1p": w1p,
            "w2p": w2p,
            "bbt": bbt,
            "obt": obt,
            "gammat": gammat,
            "betat": betat,
            "ibt": ibt,
        })

    from concourse.bass_utils import run_bass_kernel_spmd
    res = run_bass_kernel_spmd(nc, in_maps, list(range(N_CORES)))
    out = np.concatenate([res.results[c]["out"] for c in range(N_CORES)],
                         axis=0)
    return out.reshape(B, S, H)


if __name__ == "__main__":
    nc = _get_program()
    from concourse.timeline_sim import TimelineSim
    ts = TimelineSim(nc)
    total = ts.simulate()
    print(f"TimelineSim: {total:.0f} ns")


# revision 20
# speedup vs baseline: 3.9732x; 1.0308x over previous
"""DeepSpeed-style MLP block (residual-add + LayerNorm + GEMM + GeLU + GEMM +
residual) on 8 Trainium2 NeuronCores.

Sharding: data-parallel over tokens (B*S = 8192 -> 1024 tokens/core).  Each
core holds the full weights and computes its token slice end-to-end; no
collectives.

All matmuls run in bf16 on the PE (1 cycle/row vs fp32's 4) with fp32 PSUM
accumulation.  Per core the tokens are processed as two 512-token groups:
GEMM1 (64 rank-128 i-chunks) produces h^T tiles [128, 512] that stay resident
in SBUF, then GEMM2 accumulates over all 64 i-chunks into PSUM for 4 output
column chunks of 512.  Weights stream from DRAM twice (once per group), which
the DMA engines hide entirely under the PE's compute.

LayerNorm statistics use bn_stats/bn_aggr on the vector engine; the
normalized activations are transposed to [H, tok] via PE transposes with
gamma/beta applied on the PSUM->SBUF drain.
"""

import sys

sys.path.insert(0, "/opt/trn_rl_repo")

import numpy as np

try:
    import jax

    jax.config.update("jax_compilation_cache_dir", "/tmp/jax_neff_cache")
    jax.config.update("jax_persistent_cache_min_compile_time_secs", 1.0)
    jax.config.update("jax_persistent_cache_min_entry_size_bytes", 0)
except Exception:
    pass

import ml_dtypes

import concourse.bass as bass  # noqa: F401
import concourse.mybir as mybir
from concourse import bacc
from concourse.masks import make_identity
from concourse.tile import TileContext

F32 = mybir.dt.float32
BF16 = mybir.dt.bfloat16
AF = mybir.ActivationFunctionType
ALU = mybir.AluOpType
NP_BF16 = ml_dtypes.bfloat16

N_CORES = 8
B, S, H, I = 4, 2048, 2048, 8192
LN_EPS = 1e-6
NTOK = B * S                 # 8192 tokens total
TLOC = NTOK // N_CORES       # 1024 tokens per core
TT = TLOC // 128             # 8 token tiles per core
HC = H // 128                # 16 hidden chunks (contraction for GEMM1)
IC = I // 128                # 64 intermediate chunks
GROUPS = 2                   # token groups per core
GT = TT // GROUPS            # 4 token tiles per group
GTOK = TLOC // GROUPS        # 512 tokens per group
OC = H // 512                # 4 output column chunks of 512

_CACHE = {}


def _build_program():
    nc = bacc.Bacc("TRN2", target_bir_lowering=False, debug=False,
                   num_devices=N_CORES)

    xin = nc.declare_dram_parameter("xin", [TLOC, H], BF16, isOutput=False)
    xres = nc.declare_dram_parameter("xres", [TLOC, H], BF16, isOutput=False)
    # w1p[i, p, c*128 + f] = (gamma[:, None] * inter_w)[c*128 + p, i*128 + f]
    # (LayerNorm's gamma/beta are folded into W1/b1 host-side)
    w1p = nc.declare_dram_parameter("w1p", [IC, 128, H], BF16, isOutput=False)
    w2p = nc.declare_dram_parameter("w2p", [I, H], BF16, isOutput=False)
    bbt = nc.declare_dram_parameter("bbt", [128, H], BF16, isOutput=False)
    obt = nc.declare_dram_parameter("obt", [128, H], BF16, isOutput=False)
    # b1t[p, i] = (beta @ inter_w + inter_b)[i*128 + p]
    b1t = nc.declare_dram_parameter("b1t", [128, IC], F32, isOutput=False)
    out = nc.declare_dram_parameter("out", [TLOC, H], F32, isOutput=True)

    with TileContext(nc) as tc:
        with (
            tc.tile_pool(name="perm", bufs=1) as perm,
            tc.tile_pool(name="p1", bufs=2) as p1,
            tc.tile_pool(name="w1pool", bufs=3) as w1pool,
            tc.tile_pool(name="w2pool", bufs=6) as w2pool,
            tc.tile_pool(name="htpool", bufs=IC) as htpool,
            tc.tile_pool(name="osbp", bufs=5) as osbp,
            tc.tile_pool(name="ps", bufs=1, space="PSUM") as ps,
        ):
            ident = perm.tile([128, 128], BF16)
            eps = perm.tile([128, 1], F32)
            b1 = perm.tile([128, IC], F32)
            bb = perm.tile([128, H], BF16)
            ob = perm.tile([128, H], BF16)

            # ln^T, chunk-major: lnt[:, c, tok] = ln[tok, c*128 + p]
            lnta = perm.tile([128, HC, TLOC], BF16, name="lnta")
            lnt = lnta[:]
            rao = [perm.tile([128, H], BF16, name=f"rao{t}")
                   for t in range(TT)]

            zs = {}

            def p1_load(t, split=False):
                tin = p1.tile([128, H], BF16, tag="tin")
                tre = p1.tile([128, H], BF16, tag="tre")
                # issue the first tile's two loads on different queues so the
                # transfers overlap (the LayerNorm chain start gates the PE)
                eng = nc.scalar if split else nc.sync
                nc.sync.dma_start(out=tin[:], in_=xin[t * 128:(t + 1) * 128, :])
                eng.dma_start(out=tre[:], in_=xres[t * 128:(t + 1) * 128, :])
                return tin, tre

            def p1_compute(t, loaded=None):
                """residual add + LN stats + normalize for token tile t."""
                tin, tre = loaded if loaded is not None else p1_load(t)
                ra = p1.tile([128, H], BF16, tag="ra")
                nc.vector.tensor_add(ra[:], tin[:], tre[:])
                nc.vector.tensor_add(ra[:], ra[:], bb[:])
                # final-residual term (ra + output_b) off the critical path
                nc.gpsimd.tensor_add(rao[t][:], ra[:], ob[:])
                # mean/var via bn_stats over 4 chunks of 512
                stats = p1.tile([128, 4, 6], F32, tag="stats")
                rav = ra[:].rearrange("p (n f) -> p n f", f=512)
                for sub in range(4):
                    nc.vector.bn_stats(stats[:, sub, :], rav[:, sub, :])
                mv = p1.tile([128, 2], F32, tag="mv")
                nc.vector.bn_aggr(mv[:], stats[:])
                std = p1.tile([128, 1], F32, tag="std")
                nc.scalar.activation(std[:], mv[:, 1:2], AF.Sqrt, bias=eps[:])
                rstd = p1.tile([128, 1], F32, tag="rstd")
                nc.vector.reciprocal(rstd[:], std[:])
                z = p1.tile([128, H], BF16, tag="z", bufs=4)
                nc.vector.tensor_scalar(
                    z[:], ra[:], mv[:, 0:1], rstd[:],
                    op0=ALU.subtract, op1=ALU.mult)
                zs[t] = z

            def p1_transpose(t):
                """z[tok, H] -> lnt[:, c, tok]; 4 chunks per PSUM drain."""
                z = zs[t]
                for cq in range(HC // 4):
                    tr = ps.tile([128, 512], BF16, tag="trp", bufs=2)
                    for j in range(4):
                        c = cq * 4 + j
                        nc.tensor.transpose(
                            tr[:, j * 128:(j + 1) * 128],
                            z[:, c * 128:(c + 1) * 128], ident[:])
                    trv = tr[:].rearrange("p (n f) -> p n f", f=128)
                    nc.scalar.activation(
                        lnt[:, cq * 4:(cq + 1) * 4, t * 128:(t + 1) * 128],
                        trv, AF.Copy)

            hts = [[None] * IC for _ in range(GROUPS)]

            def w1_load(i):
                w1t = w1pool.tile([128, H], BF16, tag="w1t")
                nc.sync.dma_start(out=w1t[:], in_=w1p[i])
                return w1t

            def g1_chunk(g, i, w1t=None, sub=None):
                """h^T[i-block] = gelu(W1^T @ ln^T + b1) for group g.

                sub=None computes all GTOK tokens; sub=0/1 computes the
                first/second 256-token half (used to start the PE before
                the later token tiles' LayerNorm has finished).
                """
                if w1t is None:
                    w1t = w1_load(i)
                if sub is None:
                    lo, n = 0, GTOK
                else:
                    lo, n = sub * (GTOK // 2), GTOK // 2
                psh = ps.tile([128, GTOK], F32, tag="psh", bufs=2)
                for c in range(HC):
                    nc.tensor.matmul(
                        psh[:, :n],
                        w1t[:, c * 128:(c + 1) * 128],
                        lnt[:, c, g * GTOK + lo:g * GTOK + lo + n],
                        start=(c == 0), stop=(c == HC - 1))
                if sub in (None, 0):
                    ht = htpool.tile([128, GTOK], BF16, tag="ht")
                    hts[g][i] = ht
                nc.scalar.activation(hts[g][i][:, lo:lo + n], psh[:, :n],
                                     AF.Gelu, bias=b1[:, i:i + 1])

            def w2_load(oc, i):
                w2c = w2pool.tile([128, 512], BF16, tag="w2c")
                nc.scalar.dma_start(
                    out=w2c[:],
                    in_=w2p[i * 128:(i + 1) * 128, oc * 512:(oc + 1) * 512])
                return w2c

            # GEMM2 PSUM accumulators rotate through 6 tag slots (4 dedicated
            # + the GEMM1/transpose banks, idle during a GEMM2 pass) so a new
            # column pass never waits on the previous pass's drains.
            pso_slots = [("pso0", 1), ("pso1", 1), ("pso2", 1), ("pso3", 1),
                         ("psh", 2), ("trp", 2)]
            pso_cnt = [0]

            def g2_group(g, preloaded=()):
                """out[group tokens] = h @ W2 + (ra + output_b)."""
                for oc in range(OC):
                    psos = []
                    for t in range(GT):
                        tag, nb = pso_slots[(pso_cnt[0] + t) % len(pso_slots)]
                        psos.append(ps.tile([128, 512], F32,
                                            name=f"pso_{g}_{oc}_{t}",
                                            tag=tag, bufs=nb))
                    pso_cnt[0] += GT
                    for i in range(IC):
                        if oc == 0 and i < len(preloaded):
                            w2c = preloaded[i]
                        else:
                            w2c = w2_load(oc, i)
                        ht = hts[g][i]
                        for t in range(GT):
                            nc.tensor.matmul(
                                psos[t][:],
                                ht[:, t * 128:(t + 1) * 128],
                                w2c[:],
                                start=(i == 0), stop=(i == IC - 1))
                    last = (g == GROUPS - 1) and (oc == OC - 1)
                    for t in range(GT):
                        tt = g * GT + t
                        o_lo, o_n = oc * 512, 512
                        if not last:
                            osb = osbp.tile([128, 512], F32, tag="osb")
                            nc.vector.tensor_add(
                                osb[:], psos[t][:],
                                rao[tt][:, o_lo:o_lo + o_n])
                            nc.sync.dma_start(
                                out=out[tt * 128:(tt + 1) * 128,
                                        o_lo:o_lo + o_n],
                                in_=osb[:])
                        else:
                            # final pass: fan the output DMAs across SP/ACT
                            # so the flush tail is short
                            osb = osbp.tile([128, 512], F32, tag="osb")
                            nc.vector.tensor_add(
                                osb[:], psos[t][:],
                                rao[tt][:, o_lo:o_lo + o_n])
                            eng = nc.sync if t % 2 == 0 else nc.scalar
                            eng.dma_start(
                                out=out[tt * 128:(tt + 1) * 128,
                                        o_lo:o_lo + o_n],
                                in_=osb[:])

            # ---- emission order: pipeline phase 1 under GEMM1 of group 0 ----
            # DMA order puts tile 0/1 activations first so the LayerNorm
            # chain (the critical path to the first matmul) starts ASAP.
            NSUB = 8    # leading GEMM1 i-chunks run as two 256-token passes
            l0 = p1_load(0, split=True)
            nc.sync.dma_start(out=bb[:], in_=bbt[:])
            l1 = p1_load(1, split=True)
            nc.sync.dma_start(out=ob[:], in_=obt[:])
            nc.sync.dma_start(out=b1[:], in_=b1t[:])
            make_identity(nc, ident[:])
            nc.vector.memset(eps[:], LN_EPS)
            p1_compute(0, l0)
            p1_compute(1, l1)
            p1_transpose(0)
            p1_compute(2)
            p1_transpose(1)
            p1_compute(3)

            # first NSUB chunks: tokens 0-255 only (needs just tiles 0-1), so
            # the PE starts as soon as the first two LayerNorm tiles are done
            w1_first = [w1_load(i) for i in range(min(3, NSUB))]
            for i in range(0, NSUB):
                g1_chunk(0, i, w1t=w1_first[i] if i < len(w1_first) else None,
                         sub=0)
            p1_transpose(2)
            p1_transpose(3)
            p1_compute(4)
            # second half of the leading chunks (tokens 256-511)
            for i in range(0, NSUB):
                g1_chunk(0, i, sub=1)
            p1_compute(5)
            for i in range(NSUB, 16):
                g1_chunk(0, i)
            p1_transpose(4)
            p1_compute(6)
            for i in range(16, 24):
                g1_chunk(0, i)
            p1_transpose(5)
            p1_compute(7)
            for i in range(24, 32):
                g1_chunk(0, i)
            p1_transpose(6)
            for i in range(32, 40):
                g1_chunk(0, i)
            p1_transpose(7)
            for i in range(40, IC - 8):
                g1_chunk(0, i)
            # prefetch the first W2 column chunks (ACT queue) so GEMM2 starts
            # seamlessly after GEMM1's last chunk
            w2_first = [w2_load(0, i) for i in range(4)]
            for i in range(IC - 8, IC):
                g1_chunk(0, i)

            g2_group(0, preloaded=w2_first)
            for i in range(IC - 8):
                g1_chunk(1, i)
            w2_g1 = [w2_load(0, i) for i in range(4)]
            for i in range(IC - 8, IC):
                g1_chunk(1, i)
            g2_group(1, preloaded=w2_g1)

    nc.compile()
    return nc


def _get_program():
    if "nc" not in _CACHE:
        _CACHE["nc"] = _build_program()
    return _CACHE["nc"]


def kernel(input, residual, residual_norm, bias, gamma, beta,
           inter_w, inter_b, output_w, output_b):
    nc = _get_program()

    input = np.asarray(input, dtype=np.float32)
    residual = np.asarray(residual, dtype=np.float32)
    bias = np.asarray(bias, dtype=np.float32)
    gamma = np.asarray(gamma, dtype=np.float32)
    beta = np.asarray(beta, dtype=np.float32)
    inter_w = np.asarray(inter_w, dtype=np.float32)
    inter_b = np.asarray(inter_b, dtype=np.float32)
    output_w = np.asarray(output_w, dtype=np.float32)
    output_b = np.asarray(output_b, dtype=np.float32)

    xin = np.ascontiguousarray(input.reshape(NTOK, H).astype(NP_BF16))
    xres = np.ascontiguousarray(residual.reshape(NTOK, H).astype(NP_BF16))
    # fold LayerNorm's gamma/beta into W1/b1:
    #   ln @ W1 + b1 == z @ (gamma[:,None]*W1) + (beta @ W1 + b1)
    w1f = gamma[:, None].astype(np.float32) * inter_w
    b1f = beta.astype(np.float32) @ inter_w + inter_b
    # w1p[i, p, c*128+f] = w1f[c*128+p, i*128+f]
    w1p = np.ascontiguousarray(
        w1f.reshape(HC, 128, IC, 128).transpose(2, 1, 0, 3)
        .reshape(IC, 128, H).astype(NP_BF16))
    w2p = np.ascontiguousarray(output_w.astype(NP_BF16))
    bbt = np.ascontiguousarray(
        np.broadcast_to(bias.astype(NP_BF16), (128, H)))
    obt = np.ascontiguousarray(
        np.broadcast_to(output_b.astype(NP_BF16), (128, H)))
    b1t = np.ascontiguousarray(b1f.reshape(IC, 128).T)

    in_maps = []
    for c in range(N_CORES):
        in_maps.append({
            "xin": np.ascontiguousarray(xin[c * TLOC:(c + 1) * TLOC]),
            "xres": np.ascontiguousarray(xres[c * TLOC:(c + 1) * TLOC]),
            "w1p": w1p,
            "w2p": w2p,
            "bbt": bbt,
            "obt": obt,
            "b1t": b1t,
        })

    from concourse.bass_utils import run_bass_kernel_spmd
    res = run_bass_kernel_spmd(nc, in_maps, list(range(N_CORES)))
    out = np.concatenate([res.results[c]["out"] for c in range(N_CORES)],
                         axis=0)
    return out.reshape(B, S, H)


if __name__ == "__main__":
    nc = _get_program()
    from concourse.timeline_sim import TimelineSim
    ts = TimelineSim(nc)
    total = ts.simulate()
    print(f"TimelineSim: {total:.0f} ns")
